# revision 47
# baseline (speedup 1.0000x reference)
"""Trainium2 Bass kernel for relative-position attention (nn_AttentionMechanism).

Math (per batch b):
  q,k,v = h@Wq, h@Wk, h@Wv  (biases are zero in this problem)
  scores[l,r] = (q[l].k[r] + q[l].E[l-r+1023] + k[r].E[l-r+1023]) / sqrt(64)
  out = softmax(scores) @ v @ Wd

Sharding: 8 cores = (batch b in 0..3) x (query half lh in 0..1).
Each core computes out rows [lh*512, lh*512+512) for batch b.

All matmul inputs are fp16 (1 PE cycle/row vs 4 for fp32; PSUM accumulation
stays fp32).  The 1/sqrt(64) score scaling is folded on the host into Wq, Wk
and the dist_emb window.

Per-core algorithm (T orientation: score tiles are [r partitions, l free]):
  - Host pre-transposes hidden states; qT/kT = W^T @ xT matmuls; v natural
    with a 64-wide ones block appended (gives softmax denominators for free
    as extra rows of the PV matmul output).
  - Relative-position terms need a diagonal "shear" gather E[l-r+1023], which
    no TRN2 engine can do on-chip.  Mechanism: music-transformer stride trick
    through DRAM:
      kd[r,j] = k[r].E_win[j] written with row stride 640, read back with row
        stride 639 -> the read IS rel_k^T (plain HWDGE DMA).
      qd[l,u] = q[l].E_win_rev[u] written with row stride 1536, read back with
        row stride 1535 through the HWDGE xbar transpose-DMA -> rel_q^T.
  - kd is stored fp8 e5m2 (values ~N(0, 0.011) sit below e4m3's min normal;
    the ~12% quantization lands on a term that is ~3% of the score scale).
    The qd side must stay fp16: the xbar transpose-DMA is 2/4-byte only.
  - 3-stage software pipeline over heads: iteration i runs head i's kd/qd
    matmuls + psum drains + write DMAs, head (i-1)'s shear reads, and head
    (i-2)'s score tiles, so every cross-engine edge has ~a full iteration of
    slack.  Per score tile J: rel=rq+rk on GPSIMD (the only engine with no
    PSUM access, so it gets the one SBUF-only op), rel added into the
    content-score PSUM on DVE, exp on ScalarE, PV + denominators on PE.
    The psum drains balance over ACT (kd) and DVE/ACT (qd halves); the
    per-head normalize tail is deferred one iteration so PE's broadcast
    matmul never waits on a fresh reciprocal.  The k/q projections are
    spread over iterations 0-5 and the v projection over 0-1 to fill the
    otherwise DMA-idle startup.
  - DMAs are batched (one descriptor-train per head per stream) to amortize
    the ~630ns HWDGE fixed cost per DMA instruction.
"""

import sys

sys.path.insert(0, "/opt/trn_rl_repo")

import numpy as np

import concourse.bass as bass
import concourse.mybir as mybir
import concourse.tile as tile
from concourse import bacc
from concourse.bass_utils import run_bass_kernel_spmd

FP32 = mybir.dt.float32
FP16 = mybir.dt.float16
FP8 = mybir.dt.float8e5  # e5m2: kd values ~N(0,0.011) need e5's exponent range
ADD = mybir.AluOpType.add
MULT = mybir.AluOpType.mult
EXP = mybir.ActivationFunctionType.Exp

N_CORES = 8
D, H, HD = 768, 12, 64
LQ, LK = 512, 1024
EW = 1536          # E window rows per core (= LQ + LK + pad)
KD_W = 640         # kd chunk width (639 used + 1 pad col)
QD_W = 1536        # qd row stride
SCALE = 0.35355339059327373  # 8**-0.5 folded into Wq, Wk and E on the host


def _strided_view(ap, dims, extra_offset):
    """Return a copy of `ap` with its [step,count] pairs and offset replaced."""
    v = ap.copy()
    a = v.ap
    assert len(a) == len(dims), (a, dims)
    for i, d in enumerate(dims):
        a[i] = d
    v.ap = a
    v.offset = v.offset + extra_offset
    return v


def build_nc(repeats=1):
    nc = bacc.Bacc("TRN2", target_bir_lowering=False, debug=False,
                   num_devices=N_CORES)

    hq = nc.dram_tensor("hidden_q_T", [D, LQ], FP16, kind="ExternalInput").ap()
    hkv = nc.dram_tensor("hidden_kv_T", [D, LK], FP16, kind="ExternalInput").ap()
    wq = nc.dram_tensor("Wq", [D, D], FP16, kind="ExternalInput").ap()
    wk = nc.dram_tensor("Wk", [D, D], FP16, kind="ExternalInput").ap()
    wv = nc.dram_tensor("Wv", [D, D], FP16, kind="ExternalInput").ap()
    wd = nc.dram_tensor("Wd", [D, D], FP16, kind="ExternalInput").ap()
    demb = nc.dram_tensor("demb_win_T", [HD, EW], FP16, kind="ExternalInput").ap()
    dembr = nc.dram_tensor("demb_win_rev_T", [HD, EW], FP16, kind="ExternalInput").ap()
    out = nc.dram_tensor("out", [LQ, D], FP16, kind="ExternalOutput").ap()

    with tile.TileContext(nc) as tc:
        # repeats share one scratch: the WAR hazards serialize the repeat
        # bodies so a timing delta measures true serial exec time
        qd_dram = nc.dram_tensor("qd_scratch", [H, LQ, QD_W], FP16).ap()
        kd_dram = nc.dram_tensor("kd_scratch", [H, 8, 128, KD_W], FP8).ap()
        for r in range(repeats):
            _body(nc, tc, hq, hkv, wq, wk, wv, wd, demb, dembr, out,
                  qd_dram, kd_dram)
    nc.compile()
    return nc


def _dram_block_view(dram, rows, cols, nrow_tiles):
    """[rows*nrow_tiles, cols] DRAM -> [(cols,128),(128*cols,nrow_tiles),(1,cols)]
    view matching an SBUF [128, nrow_tiles, cols] tile."""
    v = dram[0:128, 0:cols].unsqueeze(1)
    return _strided_view(v, [(cols, 128), (128 * cols, nrow_tiles), (1, cols)], 0)


def _body(nc, tc, hq, hkv, wq, wk, wv, wd, demb, dembr, out, qd_dram, kd_dram):
    with tc.tile_pool(name="const", bufs=1) as cp:
        ones_row = cp.tile([1, 64], FP16, tag="ones_row")
        nc.gpsimd.memset(ones_row[:, :], 1.0)

        eT = cp.tile([128, EW], FP16, tag="eT")    # rows 0:64 == 64:128 (replicated)
        erT = cp.tile([128, EW], FP16, tag="erT")
        kT = [cp.tile([128, LK], FP16, tag=f"kT{i}", name=f"kT{i}") for i in range(6)]
        qT = [cp.tile([128, LQ], FP16, tag=f"qT{i}", name=f"qT{i}") for i in range(6)]
        vv = [cp.tile([128, 780], FP16, tag=f"v{i}", name=f"v{i}") for i in range(8)]
        ctxT = [cp.tile([128, LQ], FP16, tag=f"ctxT{i}", name=f"ctxT{i}") for i in range(6)]

        # ---------------- Phase A: input loads (host pre-transposed) --------
        xT = cp.tile([128, 6, LK], FP16, tag="xT")
        xqT = cp.tile([128, 6, LQ], FP16, tag="xqT")
        wkt = cp.tile([128, 6, D], FP16, tag="wk")
        wqt = cp.tile([128, 6, D], FP16, tag="wq")
        wvt = cp.tile([128, 6, D], FP16, tag="wv")

        # load order = first-use order: the k projection needs xT + Wk first.
        # xT is split by token halves and Wk by column halves so the first
        # projection matmul (needs all 6 xT kk-blocks but only Wk cols 0:128)
        # starts ~6us earlier.
        xv = _dram_block_view(hkv, 128, LK, 6)
        nc.sync.dma_start(out=xT[:, :, 0:512],
                          in_=_strided_view(xv, [xv.ap[0], xv.ap[1], (1, 512)], 0))
        wv_ = _dram_block_view(wk, 128, D, 6)
        nc.sync.dma_start(out=wkt[:, :, 0:384],
                          in_=_strided_view(wv_, [wv_.ap[0], wv_.ap[1], (1, 384)], 0))
        nc.sync.dma_start(out=xT[:, :, 512:1024],
                          in_=_strided_view(xv, [xv.ap[0], xv.ap[1], (1, 512)], 512))
        nc.sync.dma_start(out=wkt[:, :, 384:768],
                          in_=_strided_view(wv_, [wv_.ap[0], wv_.ap[1], (1, 384)], 384))
        nc.sync.dma_start(out=xqT[:, :, :], in_=_dram_block_view(hq, 128, LQ, 6))
        nc.sync.dma_start(out=wqt[:, :, :], in_=_dram_block_view(wq, 128, D, 6))
        for half in range(2):
            nc.sync.dma_start(out=eT[64 * half:64 * (half + 1), :], in_=demb[:, :])
            nc.sync.dma_start(out=erT[64 * half:64 * (half + 1), :], in_=dembr[:, :])
        nc.sync.dma_start(out=wvt[:, :, :], in_=_dram_block_view(wv, 128, D, 6))

        # ---------------- Phase C: per-head attention ----------------
        with tc.tile_pool(name="psC", bufs=2, space="PSUM") as pc, \
             tc.tile_pool(name="psCS", bufs=2, space="PSUM") as pcs, \
             tc.tile_pool(name="psCTX", bufs=2, space="PSUM") as pctx, \
             tc.tile_pool(name="kdq", bufs=3) as kq_pool, \
             tc.tile_pool(name="rds", bufs=3) as rd_pool, \
             tc.tile_pool(name="rel", bufs=10) as rel_pool, \
             tc.tile_pool(name="pp", bufs=10) as p_pool, \
             tc.tile_pool(name="nrm", bufs=3) as nrm_pool:

            state = {}

            def _emit_score_reads(h):
                # all shear reads for head h in one burst; they wait only on
                # head h's DRAM writes (completed earlier on the pipe) via
                # Tile-framework semaphores, then stream into SBUF while the
                # current head's scores compute
                rq_sb = rd_pool.tile([128, 8, LQ], FP16, tag="rq_sb")
                for J in range(8):
                    qdv = _strided_view(qd_dram[h], [(QD_W - 1, LQ), (1, 128)],
                                        512 + 128 * J)
                    nc.sync.dma_start(out=rq_sb[:, J, :], in_=qdv, transpose=True)
                rk_sb = rd_pool.tile([128, 8, LQ], FP8, tag="rk_sb")
                kdv = _strided_view(kd_dram[h, 0].unsqueeze(1),
                                    [(KD_W - 1, 128), (128 * KD_W, 8), (1, LQ)], 127)
                nc.sync.dma_start(out=rk_sb[:, :, :], in_=kdv)
                state[h] = (rq_sb, rk_sb)

            def _emit_write_mms(h):
                hc, hp = h // 2, h % 2
                hr = slice(64 * hp, 64 * (hp + 1))
                kd_sb = kq_pool.tile([128, 8, KD_W], FP8, tag="kd_sb")
                qd_sb = kq_pool.tile([128, 4, 1152], FP16, tag="qd_sb")
                kdps, qdps = [], []
                for J in range(8):
                    w0 = 896 - 128 * J
                    kdp = pc.tile([128, KD_W], FP32, tag="kdqd")
                    lhsT = kT[hc][hr, 128 * J:128 * (J + 1)]
                    nc.tensor.matmul(kdp[:, 0:512], lhsT, eT[hr, w0:w0 + 512],
                                     start=True, stop=True)
                    nc.tensor.matmul(kdp[:, 512:KD_W], lhsT,
                                     eT[hr, w0 + 512:w0 + KD_W],
                                     start=True, stop=True)
                    kdps.append(kdp)
                    if J < 4:
                        I = J
                        c0 = 384 - 128 * I
                        lhsTq = qT[hc][hr, 128 * I:128 * (I + 1)]
                        qdpA = pc.tile([128, KD_W], FP32, tag="kdqd")
                        for o, w in ((0, 512), (512, 128)):
                            nc.tensor.matmul(qdpA[:, o:o + w], lhsTq,
                                             erT[hr, c0 + o:c0 + o + w],
                                             start=True, stop=True)
                        qdpB = pc.tile([128, 512], FP32, tag="kdqd")
                        nc.tensor.matmul(qdpB[:, :], lhsTq,
                                         erT[hr, c0 + KD_W:c0 + KD_W + 512],
                                         start=True, stop=True)
                        qdps.append((qdpA, qdpB))
                return kd_sb, qd_sb, kdps, qdps

            def _emit_write_copy(h, wstate, J):
                # GPSIMD cannot read PSUM, so the psum drains balance over
                # ACT (kd + half of qdB) and DVE (qdA + half of qdB);
                # GPSIMD keeps the SBUF-only rel add
                kd_sb, qd_sb, kdps, qdps = wstate
                nc.scalar.copy(kd_sb[:, J, :], kdps[J][:, :])
                if J < 4:
                    qdpA, qdpB = qdps[J]
                    nc.vector.tensor_copy(qd_sb[:, J, 0:KD_W], qdpA[:, :])
                    if J % 2 == 0:
                        nc.scalar.copy(qd_sb[:, J, KD_W:1152], qdpB[:, :])
                    else:
                        nc.vector.tensor_copy(qd_sb[:, J, KD_W:1152], qdpB[:, :])

            def _emit_write_dmas(h, wstate):
                kd_sb, qd_sb, kdps, qdps = wstate
                # qd (the longer write) first: its completion semaphore then
                # propagates under the kd write, so the rq reads that depend
                # on it never leave the DMA pipe idle
                # qd rows: row step 1536, I-step = 128*1536 - 128, col start 384
                qdw = _strided_view(qd_dram[h, 0:128, 384:384 + 1152].unsqueeze(1),
                                    [(QD_W, 128), (128 * QD_W - 128, 4), (1, 1152)], 0)
                nc.sync.dma_start(out=qdw, in_=qd_sb[:, :, :])
                kdw = _strided_view(kd_dram[h, 0].unsqueeze(1),
                                    [(KD_W, 128), (128 * KD_W, 8), (1, KD_W)], 0)
                nc.sync.dma_start(out=kdw, in_=kd_sb[:, :, :])

            def _emit_score_tile(h, J, ctxp, next_wstate):
                hc, hp = h // 2, h % 2
                hr = slice(64 * hp, 64 * (hp + 1))
                rq_sb, rk_sb = state[h]
                rel_sb = rel_pool.tile([128, LQ], FP16, tag="rel_sb")
                nc.gpsimd.tensor_tensor(rel_sb[:, :], rq_sb[:, J, :],
                                        rk_sb[:, J, :], ADD)
                csp = pcs.tile([128, LQ], FP32, tag="csp")
                nc.tensor.matmul(csp[:, :], kT[hc][hr, 128 * J:128 * (J + 1)],
                                 qT[hc][hr, :], start=True, stop=True)
                nc.vector.tensor_tensor(csp[:, :], csp[:, :], rel_sb[:, :], ADD)
                # drain copies ahead of the exp so the batched write DMAs
                # (waiting on the drains) can issue as early as possible
                if next_wstate is not None:
                    _emit_write_copy(h + 1, next_wstate, J)
                p_sb = p_pool.tile([128, LQ], FP16, tag="p_sb")
                nc.scalar.activation(p_sb[:, :], csp[:, :], EXP)
                # PV (rows 0:64) + denominators (row 64) as one matmul pass
                nc.tensor.matmul(ctxp[:, :], vv[J][:, 65 * h:65 * h + 65],
                                 p_sb[:, :], start=(J == 0), stop=(J == 7))

            def _emit_score_tail(h, ctxp):
                hc, hp = h // 2, h % 2
                hr = slice(64 * hp, 64 * (hp + 1))
                # normalize: ctxT_h = ctx' * (1/denom) broadcast over partitions
                recip = nrm_pool.tile([1, LQ], FP16, tag="recip")
                with nc.allow_low_precision(reason="denoms ~1e3; fp16 recip err ~5e-4 vs 2e-2 gate"):
                    nc.vector.reciprocal(recip[:, :], ctxp[64:65, :])
                bcp = pcs.tile([64, LQ], FP32, tag="csp")
                nc.tensor.matmul(bcp[:, :], ones_row[:, :], recip[:, :],
                                 start=True, stop=True)
                bc_sb = nrm_pool.tile([64, LQ], FP16, tag="bc_sb")
                nc.scalar.copy(bc_sb[:, :], bcp[:, :])
                nc.vector.tensor_tensor(ctxT[hc][hr, :], ctxp[0:64, :],
                                        bc_sb[:, :], MULT)

            # 3-stage software pipeline: iteration i writes head i's kd/qd
            # round trip, scores head (i-2), and finally issues head (i-1)'s
            # shear reads (all on the sync queue, AFTER this iteration's
            # write DMAs so the pipe order itself provides the slack) —
            # every cross-engine dependency gets ~a full iteration of slack
            # and no read ever head-of-line-blocks a compute queue.
            def _emit_v_proj(r):
                # one 128-token block of the v projection, interleaved into
                # iterations 0-1 to fill the otherwise DMA-idle startup
                ps = pc.tile([128, D], FP32, tag="kdqd")
                for kk in range(6):
                    for o, w in ((0, 512), (512, 256)):
                        nc.tensor.matmul(ps[:, o:o + w],
                                         xT[:, kk, 128 * r:128 * (r + 1)],
                                         wvt[:, kk, o:o + w],
                                         start=(kk == 0), stop=(kk == 5))
                nc.gpsimd.memset(vv[r][:, :], 1.0)
                vdst = vv[r][:, 0:D].rearrange("p (h e) -> p h e", e=64)
                vdst = _strided_view(vdst, [vdst.ap[0], (65, 12), (1, 64)], 0)
                nc.vector.tensor_copy(vdst, ps[:, 0:D].rearrange(
                    "p (h e) -> p h e", e=64))

            def _emit_kq_proj(m):
                # m-th 128-row block of the k and q projections, interleaved
                # into iterations 0-5 so head writes start ~20us earlier
                for wt, dst, rhs, n_tok in ((wkt, kT, xT, LK), (wqt, qT, xqT, LQ)):
                    for nh in range(n_tok // 512):
                        ps = pc.tile([128, 512], FP32, tag="kdqd")
                        for kk in range(6):
                            nc.tensor.matmul(
                                ps[:, :], wt[:, kk, 128 * m:128 * (m + 1)],
                                rhs[:, kk, 512 * nh:512 * (nh + 1)],
                                start=(kk == 0), stop=(kk == 5))
                        nc.vector.tensor_copy(dst[m][:, 512 * nh:512 * (nh + 1)],
                                              ps[:, :])

            # score tails (normalize) are deferred one iteration so the PE
            # broadcast matmul never waits on the freshly-finished reciprocal
            # reads(i-1) are issued MID-iteration: by then head (i-1)'s write
            # DMAs (issued last iteration, data-gated to its end) have run on
            # the pipe, so the read issue never sits blocked at a queue head,
            # and the data still lands ~半 an iteration before its scores.
            pending_tail = None
            for i in range(H + 3):
                if pending_tail is not None:
                    _emit_score_tail(*pending_tail)
                    pending_tail = None
                if i < 6:
                    _emit_kq_proj(i)
                wstate = _emit_write_mms(i) if i < H else None
                if 2 <= i <= H + 1:
                    ctxp = pctx.tile([65, LQ], FP32, tag="ctxp")
                    for J in range(8):
                        _emit_score_tile(i - 2, J, ctxp, wstate)
                        if J == 1 and i <= H:
                            _emit_score_reads(i - 1)
                    pending_tail = (i - 2, ctxp)
                elif wstate is not None:
                    for J in range(8):
                        _emit_write_copy(i, wstate, J)
                        if J % 2 == 0:
                            _emit_v_proj(4 * i + J // 2)
                    if 1 <= i <= H:
                        _emit_score_reads(i - 1)
                if wstate is not None:
                    _emit_write_dmas(i, wstate)

        # ---------------- Phase D: output projection ----------------
        with tc.tile_pool(name="wdld", bufs=1) as dp, \
             tc.tile_pool(name="psD", bufs=2, space="PSUM") as pd, \
             tc.tile_pool(name="oD", bufs=1) as od:
            wdt = dp.tile([128, 6, D], FP16, tag="wd")
            nc.sync.dma_start(out=wdt[:, :, :], in_=_dram_block_view(wd, 128, D, 6))
            o_sb = od.tile([128, 4, D], FP16, tag="o_sb")
            for lc in range(4):
                ps = pd.tile([128, D], FP32, tag="outp")
                for kk in range(6):
                    for o, w in ((0, 512), (512, 256)):
                        nc.tensor.matmul(ps[:, o:o + w],
                                         ctxT[kk][:, 128 * lc:128 * (lc + 1)],
                                         wdt[:, kk, o:o + w],
                                         start=(kk == 0), stop=(kk == 5))
                nc.scalar.copy(o_sb[:, lc, :], ps[:, :])
            ov = _strided_view(out[0:128, :].unsqueeze(1),
                               [(D, 128), (128 * D, 4), (1, D)], 0)
            nc.sync.dma_start(out=ov, in_=o_sb[:, :, :])


_NC_CACHE = None


def _get_nc():
    global _NC_CACHE
    if _NC_CACHE is None:
        _NC_CACHE = build_nc()
    return _NC_CACHE


def make_in_maps(hidden_states, Wq, Wk, Wv, Wd, dist_emb):
    E = np.ascontiguousarray(np.asarray(dist_emb, np.float32))
    hidden_states = np.asarray(hidden_states, np.float32)
    wq16 = np.ascontiguousarray(np.asarray(Wq, np.float32) * np.float32(SCALE)).astype(np.float16)
    wk16 = np.ascontiguousarray(np.asarray(Wk, np.float32) * np.float32(SCALE)).astype(np.float16)
    wv16 = np.ascontiguousarray(Wv).astype(np.float16)
    wd16 = np.ascontiguousarray(Wd).astype(np.float16)
    in_maps = []
    for core in range(N_CORES):
        b, lh = core // 2, core % 2
        l0 = LQ * lh
        win = np.zeros((EW, HD), np.float32)
        n = min(EW, E.shape[0] - l0)
        win[:n] = E[l0:l0 + n]
        wins = win * np.float32(SCALE)
        in_maps.append({
            "hidden_q_T": np.ascontiguousarray(hidden_states[b, l0:l0 + LQ].T).astype(np.float16),
            "hidden_kv_T": np.ascontiguousarray(hidden_states[b].T).astype(np.float16),
            "Wq": wq16, "Wk": wk16, "Wv": wv16, "Wd": wd16,
            "demb_win_T": np.ascontiguousarray(wins.T).astype(np.float16),
            "demb_win_rev_T": np.ascontiguousarray(wins[::-1].T).astype(np.float16),
        })
    return in_maps


def run(inputs, trace=False):
    """Returns (full_output [4,1024,768], BassKernelResults)."""
    nc = _get_nc()
    in_maps = make_in_maps(inputs["hidden_states"], inputs["Wq"], inputs["Wk"],
                           inputs["Wv"], inputs["Wd"], inputs["dist_emb"])
    res = run_bass_kernel_spmd(nc, in_maps, list(range(N_CORES)), trace=trace)
    full = np.zeros((4, LK, D), np.float32)
    for core in range(N_CORES):
        b, lh = core // 2, core % 2
        full[b, LQ * lh:LQ * (lh + 1)] = res.results[core]["out"].astype(np.float32)
    return full, res


def kernel(**inputs):
    full, _ = run(inputs, trace=False)
    return full


if __name__ == "__main__":
    # quick self-build check
    nc = build_nc()
    print("built ok")


# revision 48
# speedup vs baseline: 1.3957x; 1.3957x over previous
"""Trainium2 Bass kernel for relative-position attention (nn_AttentionMechanism).

Math (per batch b):
  q,k,v = h@Wq, h@Wk, h@Wv  (biases are zero in this problem)
  scores[l,r] = (q[l].k[r] + q[l].E[l-r+1023] + k[r].E[l-r+1023]) / sqrt(64)
  out = softmax(scores) @ v @ Wd

Sharding: 8 cores = (batch b in 0..3) x (query half lh in 0..1).
Each core computes out rows [lh*512, lh*512+512) for batch b.

All matmul inputs are fp16 (1 PE cycle/row vs 4 for fp32; PSUM accumulation
stays fp32).  The 1/sqrt(64) score scaling is folded on the host into Wq, Wk
and the dist_emb window.

Per-core algorithm (T orientation: score tiles are [r partitions, l free]):
  - Host pre-transposes hidden states; qT/kT = W^T @ xT matmuls; v natural
    with a 64-wide ones block appended (gives softmax denominators for free
    as extra rows of the PV matmul output).
  - Relative-position terms need a diagonal "shear" gather E[l-r+1023], which
    no TRN2 engine can do on-chip.  Mechanism: music-transformer stride trick
    through DRAM:
      kd[r,j] = k[r].E_win[j] written with row stride 640, read back with row
        stride 639 -> the read IS rel_k^T (plain HWDGE DMA).
      qd[l,u] = q[l].E_win_rev[u] written with row stride 1536, read back with
        row stride 1535 through the HWDGE xbar transpose-DMA -> rel_q^T.
  - kd is stored fp8 e5m2 (values ~N(0, 0.011) sit below e4m3's min normal;
    the ~12% quantization lands on a term that is ~3% of the score scale).
    The qd side must stay fp16: the xbar transpose-DMA is 2/4-byte only.
  - 3-stage software pipeline over heads: iteration i runs head i's kd/qd
    matmuls + psum drains + write DMAs, head (i-1)'s shear reads, and head
    (i-2)'s score tiles, so every cross-engine edge has ~a full iteration of
    slack.  Per score tile J: rel=rq+rk on GPSIMD (the only engine with no
    PSUM access, so it gets the one SBUF-only op), rel added into the
    content-score PSUM on DVE, exp on ScalarE, PV + denominators on PE.
    The psum drains balance over ACT (kd) and DVE/ACT (qd halves); the
    per-head normalize tail is deferred one iteration so PE's broadcast
    matmul never waits on a fresh reciprocal.  The k/q projections are
    spread over iterations 0-5 and the v projection over 0-1 to fill the
    otherwise DMA-idle startup.
  - DMAs are batched (one descriptor-train per head per stream) to amortize
    the ~630ns HWDGE fixed cost per DMA instruction.
"""

import sys

sys.path.insert(0, "/opt/trn_rl_repo")

import numpy as np

import concourse.bass as bass
import concourse.mybir as mybir
import concourse.tile as tile
from concourse import bacc
from concourse.bass_utils import run_bass_kernel_spmd

FP32 = mybir.dt.float32
FP16 = mybir.dt.float16
FP8 = mybir.dt.float8e5  # e5m2: kd values ~N(0,0.011) need e5's exponent range
ADD = mybir.AluOpType.add
MULT = mybir.AluOpType.mult
EXP = mybir.ActivationFunctionType.Exp

N_CORES = 8
D, H, HD = 768, 12, 64
LQ, LK = 512, 1024
EW = 1536          # E window rows per core (= LQ + LK + pad)
KD_W = 640         # kd chunk width (639 used + 1 pad col)
QD_W = 1536        # qd row stride
SCALE = 0.35355339059327373  # 8**-0.5 folded into Wq, Wk and E on the host


def _strided_view(ap, dims, extra_offset):
    """Return a copy of `ap` with its [step,count] pairs and offset replaced."""
    v = ap.copy()
    a = v.ap
    assert len(a) == len(dims), (a, dims)
    for i, d in enumerate(dims):
        a[i] = d
    v.ap = a
    v.offset = v.offset + extra_offset
    return v


def build_nc(repeats=1):
    nc = bacc.Bacc("TRN2", target_bir_lowering=False, debug=False,
                   num_devices=N_CORES)

    hq = nc.dram_tensor("hidden_q_T", [D, LQ], FP16, kind="ExternalInput").ap()
    hkv = nc.dram_tensor("hidden_kv_T", [D, LK], FP16, kind="ExternalInput").ap()
    wq = nc.dram_tensor("Wq", [D, D], FP16, kind="ExternalInput").ap()
    wk = nc.dram_tensor("Wk", [D, D], FP16, kind="ExternalInput").ap()
    wv = nc.dram_tensor("Wv", [D, D], FP16, kind="ExternalInput").ap()
    wd = nc.dram_tensor("Wd", [D, D], FP16, kind="ExternalInput").ap()
    demb = nc.dram_tensor("demb_win_T", [HD, EW], FP16, kind="ExternalInput").ap()
    dembr = nc.dram_tensor("demb_win_rev_T", [HD, EW], FP16, kind="ExternalInput").ap()
    out = nc.dram_tensor("out", [LQ, D], FP16, kind="ExternalOutput").ap()

    with tile.TileContext(nc) as tc:
        # repeats share one scratch: the WAR hazards serialize the repeat
        # bodies so a timing delta measures true serial exec time
        qd_dram = nc.dram_tensor("qd_scratch", [H, LQ, QD_W], FP16).ap()
        kd_dram = nc.dram_tensor("kd_scratch", [H, 8, 128, KD_W], FP8).ap()
        for r in range(repeats):
            _body(nc, tc, hq, hkv, wq, wk, wv, wd, demb, dembr, out,
                  qd_dram, kd_dram)
    nc.compile()
    return nc


def _dram_block_view(dram, rows, cols, nrow_tiles):
    """[rows*nrow_tiles, cols] DRAM -> [(cols,128),(128*cols,nrow_tiles),(1,cols)]
    view matching an SBUF [128, nrow_tiles, cols] tile."""
    v = dram[0:128, 0:cols].unsqueeze(1)
    return _strided_view(v, [(cols, 128), (128 * cols, nrow_tiles), (1, cols)], 0)


def _body(nc, tc, hq, hkv, wq, wk, wv, wd, demb, dembr, out, qd_dram, kd_dram):
    with tc.tile_pool(name="const", bufs=1) as cp:
        ones_row = cp.tile([1, 64], FP16, tag="ones_row")
        nc.gpsimd.memset(ones_row[:, :], 1.0)

        eT = cp.tile([128, EW], FP16, tag="eT")    # rows 0:64 == 64:128 (replicated)
        erT = cp.tile([128, EW], FP16, tag="erT")
        kT = [cp.tile([128, LK], FP16, tag=f"kT{i}", name=f"kT{i}") for i in range(6)]
        qT = [cp.tile([128, LQ], FP16, tag=f"qT{i}", name=f"qT{i}") for i in range(6)]
        vv = [cp.tile([128, 780], FP16, tag=f"v{i}", name=f"v{i}") for i in range(8)]
        ctxT = [cp.tile([128, LQ], FP16, tag=f"ctxT{i}", name=f"ctxT{i}") for i in range(6)]

        # ---------------- Phase A: input loads (host pre-transposed) --------
        xT = cp.tile([128, 6, LK], FP16, tag="xT")
        xqT = cp.tile([128, 6, LQ], FP16, tag="xqT")
        wkt = cp.tile([128, 6, D], FP16, tag="wk")
        wqt = cp.tile([128, 6, D], FP16, tag="wq")
        wvt = cp.tile([128, 6, D], FP16, tag="wv")

        # load order = first-use order: the k projection needs xT + Wk first.
        # xT is split by token halves and Wk by column halves so the first
        # projection matmul (needs all 6 xT kk-blocks but only Wk cols 0:128)
        # starts ~6us earlier.
        xv = _dram_block_view(hkv, 128, LK, 6)
        nc.sync.dma_start(out=xT[:, :, 0:512],
                          in_=_strided_view(xv, [xv.ap[0], xv.ap[1], (1, 512)], 0))
        wv_ = _dram_block_view(wk, 128, D, 6)
        nc.sync.dma_start(out=wkt[:, :, 0:384],
                          in_=_strided_view(wv_, [wv_.ap[0], wv_.ap[1], (1, 384)], 0))
        nc.sync.dma_start(out=xT[:, :, 512:1024],
                          in_=_strided_view(xv, [xv.ap[0], xv.ap[1], (1, 512)], 512))
        nc.sync.dma_start(out=wkt[:, :, 384:768],
                          in_=_strided_view(wv_, [wv_.ap[0], wv_.ap[1], (1, 384)], 384))
        nc.sync.dma_start(out=xqT[:, :, :], in_=_dram_block_view(hq, 128, LQ, 6))
        nc.sync.dma_start(out=wqt[:, :, :], in_=_dram_block_view(wq, 128, D, 6))
        for half in range(2):
            nc.sync.dma_start(out=eT[64 * half:64 * (half + 1), :], in_=demb[:, :])
            nc.sync.dma_start(out=erT[64 * half:64 * (half + 1), :], in_=dembr[:, :])
        nc.sync.dma_start(out=wvt[:, :, :], in_=_dram_block_view(wv, 128, D, 6))

        # ---------------- Phase C: per-head attention ----------------
        with tc.tile_pool(name="psC", bufs=2, space="PSUM") as pc, \
             tc.tile_pool(name="psCS", bufs=3, space="PSUM") as pcs, \
             tc.tile_pool(name="psCTX", bufs=1, space="PSUM") as pctx, \
             tc.tile_pool(name="kdq", bufs=3) as kq_pool, \
             tc.tile_pool(name="rds", bufs=3) as rd_pool, \
             tc.tile_pool(name="rel", bufs=10) as rel_pool, \
             tc.tile_pool(name="pp", bufs=10) as p_pool, \
             tc.tile_pool(name="nrm", bufs=3) as nrm_pool:

            state = {}

            def _emit_score_reads(h):
                # all shear reads for head h in one burst; they wait only on
                # head h's DRAM writes (completed earlier on the pipe) via
                # Tile-framework semaphores, then stream into SBUF while the
                # current head's scores compute
                rq_sb = rd_pool.tile([128, 8, LQ], FP16, tag="rq_sb")
                for J in range(8):
                    qdv = _strided_view(qd_dram[h], [(QD_W - 1, LQ), (1, 128)],
                                        512 + 128 * J)
                    nc.sync.dma_start(out=rq_sb[:, J, :], in_=qdv, transpose=True)
                rk_sb = rd_pool.tile([128, 8, LQ], FP8, tag="rk_sb")
                kdv = _strided_view(kd_dram[h, 0].unsqueeze(1),
                                    [(KD_W - 1, 128), (128 * KD_W, 8), (1, LQ)], 127)
                nc.sync.dma_start(out=rk_sb[:, :, :], in_=kdv)
                state[h] = (rq_sb, rk_sb)

            def _emit_write_mms(h):
                hc, hp = h // 2, h % 2
                hr = slice(64 * hp, 64 * (hp + 1))
                kd_sb = kq_pool.tile([128, 8, KD_W], FP8, tag="kd_sb")
                qd_sb = kq_pool.tile([128, 4, 1152], FP16, tag="qd_sb")
                kdps, qdps = [], []
                for J in range(8):
                    w0 = 896 - 128 * J
                    kdp = pc.tile([128, KD_W], FP32, tag="kdqd")
                    lhsT = kT[hc][hr, 128 * J:128 * (J + 1)]
                    nc.tensor.matmul(kdp[:, 0:512], lhsT, eT[hr, w0:w0 + 512],
                                     start=True, stop=True)
                    nc.tensor.matmul(kdp[:, 512:KD_W], lhsT,
                                     eT[hr, w0 + 512:w0 + KD_W],
                                     start=True, stop=True)
                    kdps.append(kdp)
                    if J < 4:
                        I = J
                        c0 = 384 - 128 * I
                        lhsTq = qT[hc][hr, 128 * I:128 * (I + 1)]
                        qdpA = pc.tile([128, KD_W], FP32, tag="kdqd")
                        for o, w in ((0, 512), (512, 128)):
                            nc.tensor.matmul(qdpA[:, o:o + w], lhsTq,
                                             erT[hr, c0 + o:c0 + o + w],
                                             start=True, stop=True)
                        qdpB = pc.tile([128, 512], FP32, tag="kdqd")
                        nc.tensor.matmul(qdpB[:, :], lhsTq,
                                         erT[hr, c0 + KD_W:c0 + KD_W + 512],
                                         start=True, stop=True)
                        qdps.append((qdpA, qdpB))
                return kd_sb, qd_sb, kdps, qdps

            def _emit_write_copy(h, wstate, J):
                # GPSIMD cannot read PSUM, so the psum drains balance over
                # ACT (kd + half of qdB) and DVE (qdA + half of qdB);
                # GPSIMD keeps the SBUF-only rel add
                kd_sb, qd_sb, kdps, qdps = wstate
                nc.scalar.copy(kd_sb[:, J, :], kdps[J][:, :])
                if J < 4:
                    qdpA, qdpB = qdps[J]
                    nc.vector.tensor_copy(qd_sb[:, J, 0:KD_W], qdpA[:, :])
                    if J % 2 == 0:
                        nc.scalar.copy(qd_sb[:, J, KD_W:1152], qdpB[:, :])
                    else:
                        nc.vector.tensor_copy(qd_sb[:, J, KD_W:1152], qdpB[:, :])

            def _emit_write_dmas(h, wstate):
                kd_sb, qd_sb, kdps, qdps = wstate
                # qd (the longer write) first: its completion semaphore then
                # propagates under the kd write, so the rq reads that depend
                # on it never leave the DMA pipe idle
                # qd rows: row step 1536, I-step = 128*1536 - 128, col start 384
                qdw = _strided_view(qd_dram[h, 0:128, 384:384 + 1152].unsqueeze(1),
                                    [(QD_W, 128), (128 * QD_W - 128, 4), (1, 1152)], 0)
                nc.sync.dma_start(out=qdw, in_=qd_sb[:, :, :])
                kdw = _strided_view(kd_dram[h, 0].unsqueeze(1),
                                    [(KD_W, 128), (128 * KD_W, 8), (1, KD_W)], 0)
                nc.sync.dma_start(out=kdw, in_=kd_sb[:, :, :])

            def _emit_score_tile(h, J, ctxp, next_wstate):
                hc, hp = h // 2, h % 2
                hr = slice(64 * hp, 64 * (hp + 1))
                rq_sb, rk_sb = state[h]
                rel_sb = rel_pool.tile([128, LQ], FP16, tag="rel_sb")
                nc.gpsimd.tensor_tensor(rel_sb[:, :], rq_sb[:, J, :],
                                        rk_sb[:, J, :], ADD)
                csp = pcs.tile([128, LQ], FP32, tag="csp")
                nc.tensor.matmul(csp[:, :], kT[hc][hr, 128 * J:128 * (J + 1)],
                                 qT[hc][hr, :], start=True, stop=True)
                nc.vector.tensor_tensor(csp[:, :], csp[:, :], rel_sb[:, :], ADD)
                # drain copies ahead of the exp so the batched write DMAs
                # (waiting on the drains) can issue as early as possible
                if next_wstate is not None:
                    _emit_write_copy(h + 1, next_wstate, J)
                p_sb = p_pool.tile([128, LQ], FP16, tag="p_sb")
                nc.scalar.activation(p_sb[:, :], csp[:, :], EXP)
                # PV (rows 0:64) + denominators (row 64) as one matmul pass
                nc.tensor.matmul(ctxp[:, :], vv[J][:, 65 * h:65 * h + 65],
                                 p_sb[:, :], start=(J == 0), stop=(J == 7))

            def _emit_score_tail(h, ctxp):
                hc, hp = h // 2, h % 2
                hr = slice(64 * hp, 64 * (hp + 1))
                # normalize: ctxT_h = ctx' * (1/denom) broadcast over partitions
                recip = nrm_pool.tile([1, LQ], FP16, tag="recip")
                with nc.allow_low_precision(reason="denoms ~1e3; fp16 recip err ~5e-4 vs 2e-2 gate"):
                    nc.vector.reciprocal(recip[:, :], ctxp[64:65, :])
                bcp = pcs.tile([64, LQ], FP32, tag="csp")
                nc.tensor.matmul(bcp[:, :], ones_row[:, :], recip[:, :],
                                 start=True, stop=True)
                bc_sb = nrm_pool.tile([64, LQ], FP16, tag="bc_sb")
                nc.scalar.copy(bc_sb[:, :], bcp[:, :])
                nc.vector.tensor_tensor(ctxT[hc][hr, :], ctxp[0:64, :],
                                        bc_sb[:, :], MULT)

            # 3-stage software pipeline: iteration i writes head i's kd/qd
            # round trip, scores head (i-2), and finally issues head (i-1)'s
            # shear reads (all on the sync queue, AFTER this iteration's
            # write DMAs so the pipe order itself provides the slack) —
            # every cross-engine dependency gets ~a full iteration of slack
            # and no read ever head-of-line-blocks a compute queue.
            def _emit_v_proj(r):
                # one 128-token block of the v projection, interleaved into
                # iterations 0-1 to fill the otherwise DMA-idle startup
                ps = pc.tile([128, D], FP32, tag="kdqd")
                for kk in range(6):
                    for o, w in ((0, 512), (512, 256)):
                        nc.tensor.matmul(ps[:, o:o + w],
                                         xT[:, kk, 128 * r:128 * (r + 1)],
                                         wvt[:, kk, o:o + w],
                                         start=(kk == 0), stop=(kk == 5))
                nc.gpsimd.memset(vv[r][:, :], 1.0)
                vdst = vv[r][:, 0:D].rearrange("p (h e) -> p h e", e=64)
                vdst = _strided_view(vdst, [vdst.ap[0], (65, 12), (1, 64)], 0)
                nc.vector.tensor_copy(vdst, ps[:, 0:D].rearrange(
                    "p (h e) -> p h e", e=64))

            def _emit_kq_proj(m):
                # m-th 128-row block of the k and q projections, interleaved
                # into iterations 0-5 so head writes start ~20us earlier
                for wt, dst, rhs, n_tok in ((wkt, kT, xT, LK), (wqt, qT, xqT, LQ)):
                    for nh in range(n_tok // 512):
                        ps = pc.tile([128, 512], FP32, tag="kdqd")
                        for kk in range(6):
                            nc.tensor.matmul(
                                ps[:, :], wt[:, kk, 128 * m:128 * (m + 1)],
                                rhs[:, kk, 512 * nh:512 * (nh + 1)],
                                start=(kk == 0), stop=(kk == 5))
                        nc.vector.tensor_copy(dst[m][:, 512 * nh:512 * (nh + 1)],
                                              ps[:, :])

            # score tails (normalize) are deferred one iteration so the PE
            # broadcast matmul never waits on the freshly-finished reciprocal
            # reads(i-1) are issued MID-iteration: by then head (i-1)'s write
            # DMAs (issued last iteration, data-gated to its end) have run on
            # the pipe, so the read issue never sits blocked at a queue head,
            # and the data still lands ~半 an iteration before its scores.
            pending_tail = None
            for i in range(H + 3):
                if pending_tail is not None:
                    _emit_score_tail(*pending_tail)
                    pending_tail = None
                if i < 6:
                    _emit_kq_proj(i)
                wstate = _emit_write_mms(i) if i < H else None
                if 2 <= i <= H + 1:
                    ctxp = pctx.tile([65, LQ], FP32, tag="ctxp")
                    for J in range(8):
                        _emit_score_tile(i - 2, J, ctxp, wstate)
                        if J == 1 and i <= H:
                            _emit_score_reads(i - 1)
                    pending_tail = (i - 2, ctxp)
                elif wstate is not None:
                    for J in range(8):
                        _emit_write_copy(i, wstate, J)
                        if J % 2 == 0:
                            _emit_v_proj(4 * i + J // 2)
                    if 1 <= i <= H:
                        _emit_score_reads(i - 1)
                if wstate is not None:
                    _emit_write_dmas(i, wstate)

        # ---------------- Phase D: output projection ----------------
        with tc.tile_pool(name="wdld", bufs=1) as dp, \
             tc.tile_pool(name="psD", bufs=2, space="PSUM") as pd, \
             tc.tile_pool(name="oD", bufs=1) as od:
            wdt = dp.tile([128, 6, D], FP16, tag="wd")
            nc.sync.dma_start(out=wdt[:, :, :], in_=_dram_block_view(wd, 128, D, 6))
            o_sb = od.tile([128, 4, D], FP16, tag="o_sb")
            for lc in range(4):
                ps = pd.tile([128, D], FP32, tag="outp")
                for kk in range(6):
                    for o, w in ((0, 512), (512, 256)):
                        nc.tensor.matmul(ps[:, o:o + w],
                                         ctxT[kk][:, 128 * lc:128 * (lc + 1)],
                                         wdt[:, kk, o:o + w],
                                         start=(kk == 0), stop=(kk == 5))
                nc.scalar.copy(o_sb[:, lc, :], ps[:, :])
            ov = _strided_view(out[0:128, :].unsqueeze(1),
                               [(D, 128), (128 * D, 4), (1, D)], 0)
            nc.sync.dma_start(out=ov, in_=o_sb[:, :, :])


_NC_CACHE = None


def _get_nc():
    global _NC_CACHE
    if _NC_CACHE is None:
        _NC_CACHE = build_nc()
    return _NC_CACHE


def make_in_maps(hidden_states, Wq, Wk, Wv, Wd, dist_emb):
    E = np.ascontiguousarray(np.asarray(dist_emb, np.float32))
    hidden_states = np.asarray(hidden_states, np.float32)
    wq16 = np.ascontiguousarray(np.asarray(Wq, np.float32) * np.float32(SCALE)).astype(np.float16)
    wk16 = np.ascontiguousarray(np.asarray(Wk, np.float32) * np.float32(SCALE)).astype(np.float16)
    wv16 = np.ascontiguousarray(Wv).astype(np.float16)
    wd16 = np.ascontiguousarray(Wd).astype(np.float16)
    in_maps = []
    for core in range(N_CORES):
        b, lh = core // 2, core % 2
        l0 = LQ * lh
        win = np.zeros((EW, HD), np.float32)
        n = min(EW, E.shape[0] - l0)
        win[:n] = E[l0:l0 + n]
        wins = win * np.float32(SCALE)
        in_maps.append({
            "hidden_q_T": np.ascontiguousarray(hidden_states[b, l0:l0 + LQ].T).astype(np.float16),
            "hidden_kv_T": np.ascontiguousarray(hidden_states[b].T).astype(np.float16),
            "Wq": wq16, "Wk": wk16, "Wv": wv16, "Wd": wd16,
            "demb_win_T": np.ascontiguousarray(wins.T).astype(np.float16),
            "demb_win_rev_T": np.ascontiguousarray(wins[::-1].T).astype(np.float16),
        })
    return in_maps


def run(inputs, trace=False):
    """Returns (full_output [4,1024,768], BassKernelResults)."""
    nc = _get_nc()
    in_maps = make_in_maps(inputs["hidden_states"], inputs["Wq"], inputs["Wk"],
                           inputs["Wv"], inputs["Wd"], inputs["dist_emb"])
    res = run_bass_kernel_spmd(nc, in_maps, list(range(N_CORES)), trace=trace)
    full = np.zeros((4, LK, D), np.float32)
    for core in range(N_CORES):
        b, lh = core // 2, core % 2
        full[b, LQ * lh:LQ * (lh + 1)] = res.results[core]["out"].astype(np.float32)
    return full, res


def kernel(**inputs):
    full, _ = run(inputs, trace=False)
    return full


if __name__ == "__main__":
    # quick self-build check
    nc = build_nc()
    print("built ok")
